# revision 1
# baseline (speedup 1.0000x reference)
"""Trainium2 Bass kernel for per-expert 2-layer MLP (grouped GEMM -> GELU -> grouped GEMM).

reference: hidden = einsum('end,edh->enh', x, w1); gelu(erf); out = einsum('enh,ehd->end', h, w2)
shapes:    x [16, 2048, 1024] f32, w1 [16, 1024, 4096] f32, w2 [16, 4096, 1024] f32

Expert-parallel over 8 NeuronCores: core c owns experts [2c, 2c+1], no
cross-core communication.  Per core, per expert:

  phase A:  actT[h, n] = gelu(w1[d, h].T @ xT[d, n])   (PE matmul, contraction d)
  phase B:  out[n, d'] = actT[h, n].T @ w2[h, d']      (PE matmul, contraction h)

Layout trick: matmul1 with w1 as the stationary operand directly yields
hidden TRANSPOSED ([h, n]) which is exactly the lhsT layout matmul2 needs.
x is pre-transposed (and pre-cast to fp16, like the weights) on the host as
part of sharding, so every device-side DMA is a natural contiguous load and
the PE does nothing but the 4096 productive matmuls.  Matmuls run in fp16
with fp32 PSUM accumulation; GELU (erf) runs on ScalarE out of PSUM.
"""

import os
import sys

import numpy as np

for _p in ("/opt/trn_rl_repo", "/root/.axon_site/_ro/trn_rl_repo"):
    if os.path.isdir(_p) and _p not in sys.path:
        sys.path.append(_p)

import concourse.bacc as bacc
import concourse.tile as tile
from concourse import mybir
from concourse.bass_utils import run_bass_kernel_spmd

E, N, D, H = 16, 2048, 1024, 4096
NCORES = 8
EPC = E // NCORES        # experts per core                     = 2
P = 128                  # SBUF partitions
FD = 512                 # matmul moving free dim
NB = 512                 # token block per phase-A/B iteration
N_BLOCKS = N // NB       # = 4
N_SUB = NB // P          # row sub-blocks per token block       = 4
KD = D // P              # d-blocks (contraction of matmul 1)   = 8
KH = H // P              # h-blocks (contraction of matmul 2)   = 32
DC = D // FD             # d' chunks (free dim of matmul 2)     = 2
F16 = mybir.dt.float16
F32 = mybir.dt.float32

_CACHE = {}


def _build():
    nc = bacc.Bacc(None, target_bir_lowering=False)
    xt_d = nc.declare_dram_parameter("xt", [EPC, D, N], F16, isOutput=False)
    w1_d = nc.declare_dram_parameter("w1", [EPC, D, H], F16, isOutput=False)
    w2_d = nc.declare_dram_parameter("w2", [EPC, H, D], F16, isOutput=False)
    out_d = nc.declare_dram_parameter("out", [EPC, N, D], F32, isOutput=True)

    with (
        tile.TileContext(nc) as tc,
        tc.tile_pool(name="w1sb", bufs=1) as w1_pool,
        tc.tile_pool(name="w2sb", bufs=1) as w2_pool,
        tc.tile_pool(name="xT", bufs=2) as xt_pool,
        tc.tile_pool(name="actT", bufs=1) as act_pool,
        tc.tile_pool(name="osb", bufs=3) as out_pool,
        tc.tile_pool(name="ps_1", bufs=4, space="PSUM") as ps1_pool,
        tc.tile_pool(name="ps_2", bufs=4, space="PSUM") as ps2_pool,
    ):

        def emit_w1_loads(e):
            """4 batched strided DMAs, column-chunk-major: phase A's first
            h-blocks unblock after one 2MB chunk, and few triggers keep the
            HWDGE queue free (each dma_start costs ~0.6us of queue time)."""
            w1_sb = w1_pool.tile([P, KD, H], F16, tag="w1")
            w1_view = w1_d[e].rearrange("(k p) h -> p k h", p=P)
            # tiny first slice so phase A's first h-block unblocks ASAP
            bounds = [0, P, 1024, 2048, 3072, H]
            for lo, hi in zip(bounds, bounds[1:]):
                nc.scalar.dma_start(
                    out=w1_sb[:, :, lo:hi], in_=w1_view[:, :, lo:hi]
                )
            return w1_sb

        def emit_w2_loads(e):
            w2_sb = w2_pool.tile([P, KH, D], F16, tag="w2")
            w2_view = w2_d[e].rearrange("(h p) d -> p h d", p=P)
            HB = KH // 4
            for c in range(4):
                nc.scalar.dma_start(
                    out=w2_sb[:, c * HB : (c + 1) * HB, :],
                    in_=w2_view[:, c * HB : (c + 1) * HB, :],
                )
            return w2_sb

        def emit_x_loads(e, nb):
            n0 = nb * NB
            xt_sb = xt_pool.tile([P, KD, NB], F16, tag="xT")
            xt_view = xt_d[e].rearrange("(k p) n -> p k n", p=P)
            nc.sync.dma_start(out=xt_sb[:, :, :], in_=xt_view[:, :, n0 : n0 + NB])
            return xt_sb

        def emit_phase_a(w1_sb, xt_sb):
            actT = act_pool.tile([P, KH, NB], F16, tag="actT")
            for h in range(KH):
                ps1 = ps1_pool.tile([P, NB], F32, tag="ps1")
                for k in range(KD):
                    nc.tensor.matmul(
                        ps1,
                        lhsT=w1_sb[:, k, h * P : (h + 1) * P],
                        rhs=xt_sb[:, k, :],
                        start=(k == 0),
                        stop=(k == KD - 1),
                    )
                nc.scalar.activation(actT[:, h, :], ps1, mybir.ActivationFunctionType.Gelu)
            return actT

        def emit_phase_b(e, nb, actT, w2_sb):
            n0 = nb * NB
            for s in range(N_SUB):
                osb = out_pool.tile([P, D], F32, tag="osb")
                for c in range(DC):
                    ps2 = ps2_pool.tile([P, FD], F32, tag="ps2")
                    for h in range(KH):
                        nc.tensor.matmul(
                            ps2,
                            lhsT=actT[:, h, s * P : (s + 1) * P],
                            rhs=w2_sb[:, h, c * FD : (c + 1) * FD],
                            start=(h == 0),
                            stop=(h == KH - 1),
                        )
                    nc.vector.tensor_copy(osb[:, c * FD : (c + 1) * FD], ps2)
                nc.sync.dma_start(out=out_d[e, n0 + s * P : n0 + (s + 1) * P, :], in_=osb)

        w1_cur = emit_w1_loads(0)
        w1_next = None
        w2_cur = None
        for e in range(EPC):
            for nb in range(N_BLOCKS):
                xt_sb = emit_x_loads(e, nb)
                actT = emit_phase_a(w1_cur, xt_sb)
                if nb == 0:
                    if e == 0:
                        # Stall the w2 slot until phase A is underway: its 8MB
                        # stream otherwise saturates the paired-core HBM window
                        # (~680 of 716 GB/s) and starves the w1 chunk stream.
                        gate = w2_pool.tile([P, 4], F32, tag="w2")
                        nc.vector.tensor_copy(gate, actT[:, 4, 0:4])
                    w2_cur = emit_w2_loads(e)
                if nb == N_BLOCKS - 1 and e + 1 < EPC:
                    w1_next = emit_w1_loads(e + 1)
                emit_phase_b(e, nb, actT, w2_cur)
            w1_cur = w1_next

    nc.compile()
    return nc


def _get_nc():
    if "nc" not in _CACHE:
        _CACHE["nc"] = _build()
    return _CACHE["nc"]


def _run(inputs, trace=False, trace_cores=None):
    x = np.asarray(inputs["x"], dtype=np.float32).astype(np.float16)
    w1 = np.asarray(inputs["w1"], dtype=np.float32).astype(np.float16)
    w2 = np.asarray(inputs["w2"], dtype=np.float32).astype(np.float16)
    xt = np.ascontiguousarray(np.swapaxes(x, 1, 2))  # [E, D, N]
    nc = _get_nc()
    in_maps = [
        {
            "xt": xt[c * EPC : (c + 1) * EPC],
            "w1": np.ascontiguousarray(w1[c * EPC : (c + 1) * EPC]),
            "w2": np.ascontiguousarray(w2[c * EPC : (c + 1) * EPC]),
        }
        for c in range(NCORES)
    ]
    res = run_bass_kernel_spmd(
        nc, in_maps, list(range(NCORES)), trace=trace, trace_cores=trace_cores
    )
    out = np.concatenate([res.results[c]["out"] for c in range(NCORES)], axis=0)
    return out.astype(np.float32, copy=False), res


def kernel(**inputs) -> np.ndarray:
    out, _ = _run(inputs, trace=False)
    return out



# revision 3
# speedup vs baseline: 1.0086x; 1.0086x over previous
"""Trainium2 Bass kernel for per-expert 2-layer MLP (grouped GEMM -> GELU -> grouped GEMM).

reference: hidden = einsum('end,edh->enh', x, w1); gelu(erf); out = einsum('enh,ehd->end', h, w2)
shapes:    x [16, 2048, 1024] f32, w1 [16, 1024, 4096] f32, w2 [16, 4096, 1024] f32

Expert-parallel over 8 NeuronCores: core c owns experts [2c, 2c+1], no
cross-core communication.  Per core, per expert:

  phase A:  actT[h, n] = gelu(w1[d, h].T @ xT[d, n])   (PE matmul, contraction d)
  phase B:  out[n, d'] = actT[h, n].T @ w2[h, d']      (PE matmul, contraction h)

Matmul1 with w1 stationary directly yields hidden TRANSPOSED ([h, n]), which is
exactly the lhsT layout matmul2 needs.  All operands are pre-cast to fp16 and
pre-permuted on the host so that every device DMA moves 128 fat contiguous
per-partition segments (2-16KB descriptors):

  w1 host layout [P, HB, KD, 128]: line p = w1[k*128+p, hb*128+c], hb-major.
    An hb-range DMA is 128 x (range*2KB) contiguous.
  w2 host layout [P, KH, D]:       line p = w2[h*128+p, d], h-major.
  x  host layout [NBLK, P, KD, NB]: line p = x[nb*512+n, k*128+p] transposed.

Engine queues: GpSimd triggers all weight DMAs (FIFO order doubles as the
bandwidth priority: w1-e0 chunks first, then gated w2), Sync triggers x loads,
Vector does PSUM->SBUF fp16 copies + output stores, Scalar runs only GELU.
Both phases run two interleaved PSUM accumulation chains; a short burst of
dummy matmuls warms the PE clock (DVFS) while the first DMAs land.
"""

import os
import sys

import numpy as np

for _p in ("/opt/trn_rl_repo", "/root/.axon_site/_ro/trn_rl_repo"):
    if os.path.isdir(_p) and _p not in sys.path:
        sys.path.append(_p)

import concourse.bacc as bacc
import concourse.tile as tile
from concourse import mybir
from concourse.bass_utils import run_bass_kernel_spmd

E, N, D, H = 16, 2048, 1024, 4096
NCORES = 8
EPC = E // NCORES        # experts per core                     = 2
P = 128                  # SBUF partitions
FD = 512                 # matmul moving free dim
NB = 512                 # token block per phase-A/B iteration
N_BLOCKS = N // NB       # = 4
N_SUB = NB // P          # row sub-blocks per token block       = 4
KD = D // P              # d-blocks (contraction of matmul 1)   = 8
KH = H // P              # h-blocks (contraction of matmul 2)   = 32
HB = H // P              # h-block count for w1 layout          = 32
DC = D // FD             # d' chunks (free dim of matmul 2)     = 2
NWARM = 9                # PE clock warm-up dummy matmuls
F16 = mybir.dt.float16
F32 = mybir.dt.float32

_CACHE = {}


def _build():
    nc = bacc.Bacc(None, target_bir_lowering=False)
    xt_d = nc.declare_dram_parameter("xt", [EPC, N_BLOCKS, P, KD * NB], F16, isOutput=False)
    w1_d = nc.declare_dram_parameter("w1", [EPC, P, HB * KD * P], F16, isOutput=False)
    w2_d = nc.declare_dram_parameter("w2", [EPC, P, KH * D], F16, isOutput=False)
    out_d = nc.declare_dram_parameter("out", [EPC, N, D], F16, isOutput=True)

    with (
        tile.TileContext(nc) as tc,
        tc.tile_pool(name="warm", bufs=1) as warm_pool,
        tc.tile_pool(name="w1sb", bufs=1) as w1_pool,
        tc.tile_pool(name="w2sb", bufs=1) as w2_pool,
        tc.tile_pool(name="xT", bufs=2) as xt_pool,
        tc.tile_pool(name="actT", bufs=1) as act_pool,
        tc.tile_pool(name="osb", bufs=3) as out_pool,
        tc.tile_pool(name="ps_1", bufs=4, space="PSUM") as ps1_pool,
        tc.tile_pool(name="ps_2", bufs=4, space="PSUM") as ps2_pool,
    ):

        def emit_w1_loads(e, first):
            """hb-range chunks, 128 contiguous segments each.  Fine-grained at
            the very start so phase A's first h-blocks unblock ASAP."""
            w1_sb = w1_pool.tile([P, HB, KD, P], F16, tag="w1")
            w1_view = w1_d[e].rearrange("p (hb k c) -> p hb k c", hb=HB, k=KD)
            bounds = [0, 1, 2, 4, 8, 16, 32] if first else [0, 8, 16, 24, 32]
            for lo, hi in zip(bounds, bounds[1:]):
                nc.gpsimd.dma_start(out=w1_sb[:, lo:hi], in_=w1_view[:, lo:hi])
            return w1_sb

        def emit_w2_loads(e):
            w2_sb = w2_pool.tile([P, KH, D], F16, tag="w2")
            w2_view = w2_d[e].rearrange("p (h d) -> p h d", h=KH)
            HBC = KH // 4
            for c in range(4):
                nc.gpsimd.dma_start(
                    out=w2_sb[:, c * HBC : (c + 1) * HBC, :],
                    in_=w2_view[:, c * HBC : (c + 1) * HBC, :],
                )
            return w2_sb

        def emit_x_loads(e, nb, split):
            xt_sb = xt_pool.tile([P, KD, NB], F16, tag="xT")
            xt_view = xt_d[e, nb].rearrange("p (k n) -> p k n", k=KD)
            if split:
                nc.sync.dma_start(out=xt_sb[:, 0:2, :], in_=xt_view[:, 0:2, :])
                nc.sync.dma_start(out=xt_sb[:, 2:, :], in_=xt_view[:, 2:, :])
            else:
                nc.sync.dma_start(out=xt_sb[:, :, :], in_=xt_view[:, :, :])
            return xt_sb

        def emit_warmup():
            """Dummy matmuls on a zeroed tile: ramp the PE clock while the
            first w1/x DMAs are still in flight."""
            warm = warm_pool.tile([P, NB], F16, tag="warm")
            nc.vector.memset(warm, 0.0)
            for _ in range(NWARM):
                pw = ps1_pool.tile([P, NB], F32, tag="ps1")
                nc.tensor.matmul(pw, lhsT=warm[:, 0:P], rhs=warm, start=True, stop=True)

        def emit_phase_a(w1_sb, xt_sb):
            actT = act_pool.tile([P, KH, NB], F16, tag="actT")
            for hp in range(KH // 2):
                h0, h1 = 2 * hp, 2 * hp + 1
                ps_a = ps1_pool.tile([P, NB], F32, tag="ps1")
                ps_b = ps1_pool.tile([P, NB], F32, tag="ps1")
                for k in range(KD):
                    nc.tensor.matmul(
                        ps_a, lhsT=w1_sb[:, h0, k, :], rhs=xt_sb[:, k, :],
                        start=(k == 0), stop=(k == KD - 1),
                    )
                    nc.tensor.matmul(
                        ps_b, lhsT=w1_sb[:, h1, k, :], rhs=xt_sb[:, k, :],
                        start=(k == 0), stop=(k == KD - 1),
                    )
                nc.scalar.activation(actT[:, h0, :], ps_a, mybir.ActivationFunctionType.Gelu)
                nc.scalar.activation(actT[:, h1, :], ps_b, mybir.ActivationFunctionType.Gelu)
            return actT

        def emit_phase_b(e, nb, actT, w2_sb):
            n0 = nb * NB
            for s in range(N_SUB):
                ps_c0 = ps2_pool.tile([P, FD], F32, tag="ps2")
                ps_c1 = ps2_pool.tile([P, FD], F32, tag="ps2")
                for h in range(KH):
                    lhs = actT[:, h, s * P : (s + 1) * P]
                    nc.tensor.matmul(ps_c0, lhsT=lhs, rhs=w2_sb[:, h, 0:FD],
                                     start=(h == 0), stop=(h == KH - 1))
                    nc.tensor.matmul(ps_c1, lhsT=lhs, rhs=w2_sb[:, h, FD:D],
                                     start=(h == 0), stop=(h == KH - 1))
                osb = out_pool.tile([P, D], F16, tag="osb")
                nc.vector.tensor_copy(osb[:, 0:FD], ps_c0)
                nc.vector.tensor_copy(osb[:, FD:D], ps_c1)
                nc.sync.dma_start(
                    out=out_d[e, n0 + s * P : n0 + (s + 1) * P, :], in_=osb
                )

        emit_warmup()
        w1_cur = emit_w1_loads(0, first=True)
        w1_next = None
        w2_cur = None
        for e in range(EPC):
            for nb in range(N_BLOCKS):
                xt_sb = emit_x_loads(e, nb, split=(e == 0 and nb == 0))
                actT = emit_phase_a(w1_cur, xt_sb)
                if nb == 0:
                    if e == 0:
                        # Stall the w2 stream (same GpSimd ring, FIFO) until
                        # phase A is underway so the critical w1 stream keeps
                        # the HBM window to itself.
                        gate = w2_pool.tile([P, 4], F32, tag="w2")
                        nc.gpsimd.tensor_copy(gate, actT[:, 4, 0:4])
                    w2_cur = emit_w2_loads(e)
                if nb == N_BLOCKS - 1 and e + 1 < EPC:
                    w1_next = emit_w1_loads(e + 1, first=False)
                emit_phase_b(e, nb, actT, w2_cur)
            w1_cur = w1_next

    nc.compile()
    return nc


def _get_nc():
    if "nc" not in _CACHE:
        _CACHE["nc"] = _build()
    return _CACHE["nc"]


def _prep(inputs):
    x = np.asarray(inputs["x"], dtype=np.float32).astype(np.float16)
    w1 = np.asarray(inputs["w1"], dtype=np.float32).astype(np.float16)
    w2 = np.asarray(inputs["w2"], dtype=np.float32).astype(np.float16)
    # x [E,N,D] -> [E, NBLK, P, KD*NB]; line p = x[nb*512+n', k*128+p]
    xt = np.ascontiguousarray(
        x.reshape(E, N_BLOCKS, NB, KD, P).transpose(0, 1, 4, 3, 2)
    ).reshape(E, N_BLOCKS, P, KD * NB)
    # w1 [E,D,H] -> [E, P, HB*KD*128]; line p = w1[k*128+p, hb*128+c], hb-major
    w1p = np.ascontiguousarray(
        w1.reshape(E, KD, P, HB, P).transpose(0, 2, 3, 1, 4)
    ).reshape(E, P, HB * KD * P)
    # w2 [E,H,D] -> [E, P, KH*D]; line p = w2[h*128+p, d], h-major
    w2p = np.ascontiguousarray(
        w2.reshape(E, KH, P, D).transpose(0, 2, 1, 3)
    ).reshape(E, P, KH * D)
    return xt, w1p, w2p


def _run(inputs, trace=False, trace_cores=None):
    xt, w1p, w2p = _prep(inputs)
    nc = _get_nc()
    in_maps = [
        {
            "xt": xt[c * EPC : (c + 1) * EPC],
            "w1": w1p[c * EPC : (c + 1) * EPC],
            "w2": w2p[c * EPC : (c + 1) * EPC],
        }
        for c in range(NCORES)
    ]
    res = run_bass_kernel_spmd(
        nc, in_maps, list(range(NCORES)), trace=trace, trace_cores=trace_cores
    )
    out = np.concatenate([res.results[c]["out"] for c in range(NCORES)], axis=0)
    return out.astype(np.float32), res


def kernel(**inputs) -> np.ndarray:
    out, _ = _run(inputs, trace=False)
    return out


# revision 8
# speedup vs baseline: 1.0101x; 1.0015x over previous
"""Trainium2 Bass kernel for per-expert 2-layer MLP (grouped GEMM -> GELU -> grouped GEMM).

reference: hidden = einsum('end,edh->enh', x, w1); gelu(erf); out = einsum('enh,ehd->end', h, w2)
shapes:    x [16, 2048, 1024] f32, w1 [16, 1024, 4096] f32, w2 [16, 4096, 1024] f32

Expert-parallel over 8 NeuronCores: core c owns experts [2c, 2c+1], no
cross-core communication.  Per core, per expert:

  phase A:  actT[h, n] = gelu(w1[d, h].T @ xT[d, n])   (PE matmul, contraction d)
  phase B:  out[n, d'] = actT[h, n].T @ w2[h, d']      (PE matmul, contraction h)

Matmul1 with w1 stationary directly yields hidden TRANSPOSED ([h, n]), which is
exactly the lhsT layout matmul2 needs.  All operands are pre-cast to fp16 and
pre-permuted on the host so that every device DMA moves 128 fat contiguous
per-partition segments (2-16KB descriptors):

  w1 host layout [P, HB, KD, 128]: line p = w1[k*128+p, hb*128+c], hb-major.
    An hb-range DMA is 128 x (range*2KB) contiguous.
  w2 host layout [P, KH, D]:       line p = w2[h*128+p, d], h-major.
  x  host layout [NBLK, P, KD, NB]: line p = x[nb*512+n, k*128+p] transposed.

Engine queues: GpSimd triggers all weight DMAs (FIFO order doubles as the
bandwidth priority: w1-e0 chunks first, then gated w2), Sync triggers x loads,
Vector does PSUM->SBUF fp16 copies + output stores, Scalar runs only GELU.
Both phases run two interleaved PSUM accumulation chains; a short burst of
dummy matmuls warms the PE clock (DVFS) while the first DMAs land.
"""

import os
import sys

import numpy as np

for _p in ("/opt/trn_rl_repo", "/root/.axon_site/_ro/trn_rl_repo"):
    if os.path.isdir(_p) and _p not in sys.path:
        sys.path.append(_p)

import concourse.bacc as bacc
import concourse.tile as tile
from concourse import mybir
from concourse.bass_utils import run_bass_kernel_spmd

E, N, D, H = 16, 2048, 1024, 4096
NCORES = 8
EPC = E // NCORES        # experts per core                     = 2
P = 128                  # SBUF partitions
FD = 512                 # matmul moving free dim
NB = 512                 # token block per phase-A/B iteration
N_BLOCKS = N // NB       # = 4
N_SUB = NB // P          # row sub-blocks per token block       = 4
KD = D // P              # d-blocks (contraction of matmul 1)   = 8
KH = H // P              # h-blocks (contraction of matmul 2)   = 32
HB = H // P              # h-block count for w1 layout          = 32
DC = D // FD             # d' chunks (free dim of matmul 2)     = 2
NWARM = 8                # PE clock warm-up dummy matmuls
F16 = mybir.dt.float16
F32 = mybir.dt.float32

_CACHE = {}


def _build():
    nc = bacc.Bacc(None, target_bir_lowering=False)
    xt_d = nc.declare_dram_parameter("xt", [EPC, N_BLOCKS, P, KD * NB], F16, isOutput=False)
    w1_d = nc.declare_dram_parameter("w1", [EPC, P, HB * KD * P], F16, isOutput=False)
    w2_d = nc.declare_dram_parameter("w2", [EPC, P, KH * D], F16, isOutput=False)
    out_d = nc.declare_dram_parameter("out", [EPC, N, D], F16, isOutput=True)

    with (
        tile.TileContext(nc) as tc,
        tc.tile_pool(name="warm", bufs=1) as warm_pool,
        tc.tile_pool(name="w1sb", bufs=1) as w1_pool,
        tc.tile_pool(name="w2sb", bufs=1) as w2_pool,
        tc.tile_pool(name="xT", bufs=2) as xt_pool,
        tc.tile_pool(name="actT", bufs=1) as act_pool,
        tc.tile_pool(name="osb", bufs=3) as out_pool,
        tc.tile_pool(name="ps_1", bufs=4, space="PSUM") as ps1_pool,
        tc.tile_pool(name="ps_2", bufs=4, space="PSUM") as ps2_pool,
    ):

        def emit_w1_loads(e, first):
            """hb-range chunks, 128 contiguous segments each.  Fine-grained at
            the very start so phase A's first h-blocks unblock ASAP."""
            w1_sb = w1_pool.tile([P, HB, KD, P], F16, tag="w1")
            w1_view = w1_d[e].rearrange("p (hb k c) -> p hb k c", hb=HB, k=KD)
            bounds = [0, 1, 2, 4, 8, 16, 32] if first else [0, 8, 16, 24, 32]
            for lo, hi in zip(bounds, bounds[1:]):
                nc.gpsimd.dma_start(out=w1_sb[:, lo:hi], in_=w1_view[:, lo:hi])
            return w1_sb

        def emit_w2_loads(e):
            w2_sb = w2_pool.tile([P, KH, D], F16, tag="w2")
            w2_view = w2_d[e].rearrange("p (h d) -> p h d", h=KH)
            HBC = KH // 4
            for c in range(4):
                nc.gpsimd.dma_start(
                    out=w2_sb[:, c * HBC : (c + 1) * HBC, :],
                    in_=w2_view[:, c * HBC : (c + 1) * HBC, :],
                )
            return w2_sb

        def emit_x_loads(e, nb, split):
            xt_sb = xt_pool.tile([P, KD, NB], F16, tag="xT")
            xt_view = xt_d[e, nb].rearrange("p (k n) -> p k n", k=KD)
            if split:
                nc.sync.dma_start(out=xt_sb[:, 0:2, :], in_=xt_view[:, 0:2, :])
                nc.sync.dma_start(out=xt_sb[:, 2:, :], in_=xt_view[:, 2:, :])
            else:
                nc.sync.dma_start(out=xt_sb[:, :, :], in_=xt_view[:, :, :])
            return xt_sb

        def emit_warmup():
            """Dummy matmuls on a zeroed tile: ramp the PE clock (DVFS takes
            ~3us of continuous execution) while the first w1/x DMAs land."""
            warm = warm_pool.tile([P, NB], F16, tag="warm")
            nc.gpsimd.memset(warm, 0.0)
            for _ in range(NWARM):
                pw = ps1_pool.tile([P, NB], F32, tag="ps1")
                nc.tensor.matmul(pw, lhsT=warm[:, 0:P], rhs=warm, start=True, stop=True)

        def emit_phase_a(w1_sb, xt_sb):
            actT = act_pool.tile([P, KH, NB], F16, tag="actT")
            for h in range(KH):
                ps1 = ps1_pool.tile([P, NB], F32, tag="ps1")
                for k in range(KD):
                    nc.tensor.matmul(
                        ps1, lhsT=w1_sb[:, h, k, :], rhs=xt_sb[:, k, :],
                        start=(k == 0), stop=(k == KD - 1),
                    )
                nc.scalar.activation(actT[:, h, :], ps1, mybir.ActivationFunctionType.Gelu)
            return actT

        def emit_phase_b(e, nb, actT, w2_sb, last):
            n0 = nb * NB
            for s in range(N_SUB):
                osb = out_pool.tile([P, D], F16, tag="osb")
                split = last and s == N_SUB - 1
                for c in range(DC):
                    ps2 = ps2_pool.tile([P, FD], F32, tag="ps2")
                    for h in range(KH):
                        nc.tensor.matmul(
                            ps2, lhsT=actT[:, h, s * P : (s + 1) * P],
                            rhs=w2_sb[:, h, c * FD : (c + 1) * FD],
                            start=(h == 0), stop=(h == KH - 1),
                        )
                    nc.vector.tensor_copy(osb[:, c * FD : (c + 1) * FD], ps2)
                    if split:
                        # tail: overlap the c=1 chain + cast with the c=0 store
                        nc.sync.dma_start(
                            out=out_d[e, n0 + s * P : n0 + (s + 1) * P,
                                      c * FD : (c + 1) * FD],
                            in_=osb[:, c * FD : (c + 1) * FD],
                        )
                if not split:
                    nc.sync.dma_start(
                        out=out_d[e, n0 + s * P : n0 + (s + 1) * P, :], in_=osb
                    )

        emit_warmup()
        w1_cur = emit_w1_loads(0, first=True)
        w1_next = None
        w2_cur = None
        for e in range(EPC):
            for nb in range(N_BLOCKS):
                xt_sb = emit_x_loads(e, nb, split=(e == 0 and nb == 0))
                actT = emit_phase_a(w1_cur, xt_sb)
                if nb == 0:
                    if e == 0:
                        # Stall the w2 stream (same GpSimd ring, FIFO) until
                        # phase A is underway so the critical w1 stream keeps
                        # the HBM window to itself.
                        gate = w2_pool.tile([P, 4], F32, tag="w2")
                        nc.gpsimd.tensor_copy(gate, actT[:, 4, 0:4])
                    w2_cur = emit_w2_loads(e)
                if nb == N_BLOCKS - 1 and e + 1 < EPC:
                    w1_next = emit_w1_loads(e + 1, first=False)
                emit_phase_b(e, nb, actT, w2_cur,
                             last=(e == EPC - 1 and nb == N_BLOCKS - 1))
            w1_cur = w1_next

    nc.compile()
    return nc


def _get_nc():
    if "nc" not in _CACHE:
        _CACHE["nc"] = _build()
    return _CACHE["nc"]


def _prep(inputs):
    x = np.asarray(inputs["x"], dtype=np.float32).astype(np.float16)
    w1 = np.asarray(inputs["w1"], dtype=np.float32).astype(np.float16)
    w2 = np.asarray(inputs["w2"], dtype=np.float32).astype(np.float16)
    # x [E,N,D] -> [E, NBLK, P, KD*NB]; line p = x[nb*512+n', k*128+p]
    xt = np.ascontiguousarray(
        x.reshape(E, N_BLOCKS, NB, KD, P).transpose(0, 1, 4, 3, 2)
    ).reshape(E, N_BLOCKS, P, KD * NB)
    # w1 [E,D,H] -> [E, P, HB*KD*128]; line p = w1[k*128+p, hb*128+c], hb-major
    w1p = np.ascontiguousarray(
        w1.reshape(E, KD, P, HB, P).transpose(0, 2, 3, 1, 4)
    ).reshape(E, P, HB * KD * P)
    # w2 [E,H,D] -> [E, P, KH*D]; line p = w2[h*128+p, d], h-major
    w2p = np.ascontiguousarray(
        w2.reshape(E, KH, P, D).transpose(0, 2, 1, 3)
    ).reshape(E, P, KH * D)
    return xt, w1p, w2p


def _run(inputs, trace=False, trace_cores=None):
    xt, w1p, w2p = _prep(inputs)
    nc = _get_nc()
    in_maps = [
        {
            "xt": xt[c * EPC : (c + 1) * EPC],
            "w1": w1p[c * EPC : (c + 1) * EPC],
            "w2": w2p[c * EPC : (c + 1) * EPC],
        }
        for c in range(NCORES)
    ]
    res = run_bass_kernel_spmd(
        nc, in_maps, list(range(NCORES)), trace=trace, trace_cores=trace_cores
    )
    out = np.concatenate([res.results[c]["out"] for c in range(NCORES)], axis=0)
    return out.astype(np.float32), res


def kernel(**inputs) -> np.ndarray:
    out, _ = _run(inputs, trace=False)
    return out
